# revision 38
# baseline (speedup 1.0000x reference)
"""Trainium2 Bass kernel for nn_Mlpmoe (moe_routing).

Structure of the problem (B=64, P=256, D=768, H=3072, 6 classes, 5+5 expert atoms):
  - patch tokens [B,256,D] go through a dense MLP (W1 -> gelu -> W2)   (~155 GFLOP)
  - 6 cls tokens  [B,6,D] each go through 2 experts (atom1 -> gelu -> atom2),
    combined with a top-1 softmax gate over 2 experts per class         (~7 GFLOP)

Sharding over 8 NeuronCores:
  - patch MLP: data-parallel over batch (8 batches/core), moe0 weights replicated,
    computed in bf16 with fp32 PSUM accumulation.
  - cls experts: top-1 routed on host (gates are exactly 0/1 after top-1+renorm;
    fp32 logit margins ~1.7e-3 >> noise) - only the SELECTED expert per
    (batch, class) is computed. Tokens are sorted by (atom1, atom2) so layer-1
    groups are contiguous; a vector-engine reorder makes layer-2 groups
    contiguous too. Hidden-dim (H) parallel across cores: core c computes all
    routed tokens for H-slice [c*384,(c+1)*384); per-core partials summed on
    host. The program is compiled per routing signature (padded bucket counts)
    and cached.

All activations/weights are fed to the device in bf16 (layouts pre-transposed on
host so no on-device transposes are needed); outputs come back fp32.

Schedule notes (from trace analysis):
  - ~7.2us of Tile/runtime preamble is fixed; DMA data starts flowing ~1.5us
    after the first trigger. The critical first loads are split fine-grained
    (xt0 halves interleaved with small w1 chunks) so L1 matmuls start ~10.5us.
  - PE warmup (HAM clock gate 1.2->2.4 GHz) via small N=128 matmuls on zeros
    sized to end right when the first real data lands.
  - w1 rest / w2 stream on the scalar HWDGE ring behind gelu slots, rationed so
    w1 chunks stay ahead of their consuming psum groups and w2 fully lands
    before tile-0 L2 needs it.
  - final psum group split in four so the last store is only 64KB.
"""

import numpy as np
import ml_dtypes

# ---------------------------------------------------------------- constants
NCORES = 8
B, PT, D, H = 64, 256, 768, 3072
NCLS = 6
KD = D // 128            # 6 contraction tiles of 128 over D
MH = H // 128            # 24 tiles over H
HS = H // NCORES         # 384 per-core hidden slice
HK = HS // 128           # 3 tiles over the slice
BPC = B // NCORES        # 8 batches per core
TPC = BPC * PT           # 2048 patch tokens per core
TN = 512                 # token tile (matmul free dim / one PSUM bank)
NT = TPC // TN           # 4 token tiles
WARMUP_N = 44            # N=128 warmup matmuls (~4.7us at cold clock)

PAIRS = [[(0, 3), (3, 0)], [(0, 4), (4, 0)], [(1, 3), (3, 1)],
         [(1, 4), (4, 1)], [(2, 3), (3, 2)], [(2, 4), (4, 2)]]
# atom1 -> possible atom2 partners (and vice versa for P2)
P1 = {0: [3, 4], 1: [3, 4], 2: [3, 4], 3: [0, 1, 2], 4: [0, 1, 2]}
P2 = {c: [a for a in range(5) if c in P1[a]] for c in range(5)}
ORD_PAIRS = [(a, c) for a in range(5) for c in P1[a]]  # 12 ordered pairs


def _cls_layout(sig):
    """Column layout for the routed cls phase, from padded bucket counts."""
    Np = dict(zip(ORD_PAIRS, sig))
    W1 = {a: sum(Np[(a, c)] for c in P1[a]) for a in range(5)}
    W2 = {c: sum(Np[(a, c)] for a in P2[c]) for c in range(5)}
    o1, l1off = {}, {}
    off = 0
    for a in range(5):
        o1[a] = off
        loc = 0
        for c in P1[a]:
            l1off[(a, c)] = loc
            loc += Np[(a, c)]
        off += W1[a]
    o2, l2off = {}, {}
    off = 0
    for c in range(5):
        o2[c] = off
        loc = 0
        for a in P2[c]:
            l2off[(a, c)] = loc
            loc += Np[(a, c)]
        off += W2[c]
    return Np, W1, W2, o1, o2, l1off, l2off, sum(sig)


_NC_CACHE = {}


def _build_nc(sig):
    """Build + bacc-compile the (SPMD, identical on all cores) Bass program."""
    if sig in _NC_CACHE:
        return _NC_CACHE[sig]

    from contextlib import ExitStack
    import concourse.bass as bass  # noqa: F401
    import concourse.mybir as mybir
    import concourse.tile as tile
    from concourse import bacc
    from concourse.tile import add_dep_helper

    f32 = mybir.dt.float32
    bf16 = mybir.dt.bfloat16
    AF = mybir.ActivationFunctionType
    ts = bass.ts

    Np, W1, W2, o1, o2, l1off, l2off, NCp = _cls_layout(sig)

    # disable_frame_to_traceback keeps python source paths out of the BIR, so
    # the compiled-NEFF cache hits no matter which directory kernel.py runs from
    nc = bacc.Bacc("TRN2", target_bir_lowering=False, debug=False,
                   enable_asserts=False, num_devices=NCORES,
                   disable_frame_to_traceback=True)

    # k-major x: each tile load is KD strided 1KB rows per partition. Do NOT
    # "optimize" this to a tile-major contiguous layout - 6KB contiguous
    # per-partition rows serialize the DMA engine fan-out and cost +50us
    # (measured); the strided rows parallelize across the 16 DMA engines.
    xp = nc.dram_tensor("xp", [128, KD, TPC], bf16, kind="ExternalInput").ap()
    # hm-major weight layout: [p, hm, k, 128] so the first L1 psum group only
    # needs the first small chunk of W1 before matmuls can start
    w1 = nc.dram_tensor("w1", [128, MH, KD, 128], bf16, kind="ExternalInput").ap()
    w2 = nc.dram_tensor("w2", [128, MH, D], bf16, kind="ExternalInput").ap()
    b1 = nc.dram_tensor("b1", [128, MH], f32, kind="ExternalInput").ap()
    b2 = nc.dram_tensor("b2", [128, KD], f32, kind="ExternalInput").ap()
    xc = nc.dram_tensor("xc", [128, KD, NCp], bf16, kind="ExternalInput").ap()
    a1 = nc.dram_tensor("a1", [5, 128, KD, HS], bf16, kind="ExternalInput").ap()
    a1b = nc.dram_tensor("a1b", [128, 5, HK], f32, kind="ExternalInput").ap()
    a2 = nc.dram_tensor("a2", [5, 128, HK, D], bf16, kind="ExternalInput").ap()
    yp = nc.dram_tensor("yp", [128, KD, TPC], f32, kind="ExternalOutput").ap()
    yc = nc.dram_tensor("yc", [128, KD, NCp], f32, kind="ExternalOutput").ap()

    with tile.TileContext(nc) as tc, ExitStack() as ctx:
        wp = ctx.enter_context(tc.tile_pool(name="weights", bufs=1))
        xpool = ctx.enter_context(tc.tile_pool(name="xin", bufs=2))
        hpool = ctx.enter_context(tc.tile_pool(name="hmid", bufs=1))
        opool = ctx.enter_context(tc.tile_pool(name="out", bufs=1))
        cpool = ctx.enter_context(tc.tile_pool(name="cls", bufs=1))
        pspool = ctx.enter_context(tc.tile_pool(name="ps", bufs=8, space="PSUM"))

        # ---- DMA schedule ------------------------------------------------
        # critical path (sync HWDGE ring): fine-grained first loads so the
        # first L1 psum group starts as soon as ~600KB has landed.
        w1t = wp.tile([128, MH, KD, 128], bf16)
        xt0 = xpool.tile([128, KD, TN], bf16, tag="xt", name="xt0")
        b1t = wp.tile([128, MH], f32)
        b2t = wp.tile([128, KD], f32)
        # single sync-ring stream in consumption order: the HWDGE rings share
        # the ~305 GB/s per-core DMA bandwidth (measured), so splitting across
        # rings only reorders arrivals - one ring in need-order is optimal.
        # 7 triggers, need-ordered: each DMA trigger costs ~0.65us of serial
        # sync-queue time before its descriptor even reaches the DGE, so the
        # critical stream wants FEW, large transfers in consumption order.
        nc.sync.dma_start(xt0[:, 0:3], xp[:, 0:3, ts(0, TN)])
        nc.sync.dma_start(w1t[:, 0:1], w1[:, 0:1])
        nc.sync.dma_start(b1t[:], b1[:])
        nc.sync.dma_start(xt0[:, 3:6], xp[:, 3:6, ts(0, TN)])
        nc.sync.dma_start(w1t[:, 1:3], w1[:, 1:3])
        nc.sync.dma_start(w1t[:, 3:6], w1[:, 3:6])
        nc.sync.dma_start(b2t[:], b2[:])

        # non-critical loads go on the scalar HWDGE ring, triggered between
        # gelu activations so they don't steal DMA bandwidth from W1/x0
        w2t = wp.tile([128, MH, D], bf16)
        xct = cpool.tile([128, KD, NCp], bf16)
        a1t = cpool.tile([128, 5, KD, HS], bf16)
        a1bt = cpool.tile([128, 5, HK], f32)
        a2t = cpool.tile([128, 5, HK, D], bf16)
        h2pre = cpool.tile([128, HK, NCp], bf16)   # gelu out, atom1-major
        h2 = cpool.tile([128, HK, NCp], bf16)      # reordered, atom2-major
        outc = cpool.tile([128, KD, NCp], f32)

        # second token tile preallocated so its load can be deferred (with
        # bufs=2 the slot is free at kernel start, so an in-loop load would be
        # hoisted into the startup window and steal bandwidth from W1/x0)
        xt1 = xpool.tile([128, KD, TN], bf16, tag="xt", name="xt1")

        # one trigger per gelu slot. Rationing: w1 chunks (393KB) at even
        # slots stay ~2 groups ahead of their consumers; w2 sixths (786KB) at
        # every 4th slot fill the leftover ring bandwidth and land before
        # tile-0 L2 consumes them; cls inputs stream during tile 1.
        _dl = {}
        for i in range(9):
            _dl[(0, 2 * i)] = lambda i=i: nc.scalar.dma_start(
                w1t[:, 6 + 2 * i:8 + 2 * i], w1[:, 6 + 2 * i:8 + 2 * i])
        for j in range(6):
            _dl[(0, 4 * j + 1)] = lambda j=j: nc.scalar.dma_start(
                w2t[:, 4 * j:4 * (j + 1)], w2[:, 4 * j:4 * (j + 1)])
        _dl[(0, 19)] = lambda: nc.scalar.dma_start(xt1[:], xp[:, :, ts(1, TN)])
        _dl[(1, 0)] = lambda: nc.scalar.dma_start(xct[:], xc[:])
        for j in range(5):
            _dl[(1, 2 + 2 * j)] = lambda j=j: nc.scalar.dma_start(a1t[:, j], a1[j])
            _dl[(1, 14 + 2 * j)] = lambda j=j: nc.scalar.dma_start(a2t[:, j], a2[j])
        _dl[(1, 12)] = lambda: nc.scalar.dma_start(a1bt[:], a1b[:])

        def _ins(x):
            return getattr(x, "ins", x)

        def deferred_loads(t, hm, act):
            fn = _dl.get((t, hm))
            if fn is not None:
                d = fn()
                # same-engine ordering edge: without it the scheduler hoists
                # the (dependency-free) trigger to kernel start, where its
                # transfer steals DMA bandwidth from the critical W1/x0 loads
                add_dep_helper(_ins(d), _ins(act), sync=False,
                               reason="defer bulk load behind gelu")

        # ---- patch MLP tile body ----------------------------------------
        def patch_tile(t, xt):
            ht = hpool.tile([128, MH, TN], bf16, tag="ht", name="ht")
            for hm in range(MH):
                pt = pspool.tile([128, TN], f32, tag="ps", name="pt")
                for k in range(KD):
                    nc.tensor.matmul(pt[:], w1t[:, hm, k, :], xt[:, k, :],
                                     start=(k == 0), stop=(k == KD - 1))
                act = nc.scalar.activation(ht[:, hm, :], pt[:], AF.Gelu,
                                           bias=b1t[:, hm, None])
                deferred_loads(t, hm, act)
            ot = opool.tile([128, KD, TN], f32, tag="ot", name="ot")
            for dm in range(KD):
                if t == NT - 1 and dm == KD - 1:
                    # final group split in four: earlier quarters' epilogues
                    # and stores overlap later quarters' matmuls, so the
                    # kernel tail only waits for the last 64 KB store
                    for hf in range(4):
                        QN = TN // 4
                        pt = pspool.tile([128, TN], f32, tag="ps", name="pt")
                        po = pt[:, :QN]
                        hsl = slice(hf * QN, (hf + 1) * QN)
                        for k in range(MH):
                            nc.tensor.matmul(po, w2t[:, k, ts(dm, 128)],
                                             ht[:, k, hsl],
                                             start=(k == 0), stop=(k == MH - 1))
                        nc.vector.tensor_scalar_add(ot[:, dm, hsl], po,
                                                    b2t[:, dm, None])
                        # final stores via the gpsimd software-DGE queue: its
                        # queue is idle here, so the ~0.6us HWDGE trigger
                        # serialization leaves the kernel-tail critical path
                        nc.gpsimd.dma_start(
                            yp[:, dm, t * TN + hf * QN:t * TN + (hf + 1) * QN],
                            ot[:, dm, hsl])
                    continue
                pt = pspool.tile([128, TN], f32, tag="ps", name="pt")
                for k in range(MH):
                    nc.tensor.matmul(pt[:], w2t[:, k, ts(dm, 128)], ht[:, k, :],
                                     start=(k == 0), stop=(k == MH - 1))
                nc.vector.tensor_scalar_add(ot[:, dm, :], pt[:], b2t[:, dm, None])
                # per-dm store so the tail only waits for the last 256 KB
                nc.sync.dma_start(yp[:, dm, ts(t, TN)], ot[:, dm, :])

        # ---- routed cls phase -------------------------------------------
        def cls_phase():
            if NCp == 0:
                return
            # layer 1: contiguous per-atom1 groups, one gelu per (a, hm).
            # a=3,4 first: layer 2 starts with c=0,1,2 whose inputs come from
            # a=3,4, so their gelus+reorders are long done when L1 ends and
            # a<3's reorder hides under L2's first groups.
            for a in (3, 4, 0, 1, 2):
                if W1[a] == 0:
                    continue
                for hm in range(HK):
                    pt = pspool.tile([128, TN], f32, tag="ps", name="pt")
                    po = pt[:, :W1[a]]
                    for k in range(KD):
                        nc.tensor.matmul(po, a1t[:, a, k, ts(hm, 128)],
                                         xct[:, k, o1[a]:o1[a] + W1[a]],
                                         start=(k == 0), stop=(k == KD - 1))
                    nc.scalar.activation(h2pre[:, hm, o1[a]:o1[a] + W1[a]], po,
                                         AF.Gelu, bias=a1bt[:, a, hm, None])
            # reorder atom1-major -> atom2-major (vector engine, idle here)
            for a in range(5):
                for c in P1[a]:
                    n = Np[(a, c)]
                    if n == 0:
                        continue
                    s1 = o1[a] + l1off[(a, c)]
                    s2 = o2[c] + l2off[(a, c)]
                    nc.vector.tensor_scalar_add(h2[:, :, s2:s2 + n],
                                                h2pre[:, :, s1:s1 + n], 0.0)
            # layer 2: contiguous per-atom2 groups; copy psum -> outc
            # (alternate vector / scalar engines so neither serializes)
            def _copy_out(n, dst, src):
                if n % 2 == 0:
                    nc.vector.tensor_scalar_add(dst, src, 0.0)
                else:
                    nc.scalar.copy(dst, src)
            for ci, c in enumerate(range(5)):
                if W2[c] == 0:
                    continue
                for dm in range(KD):
                    pt = pspool.tile([128, TN], f32, tag="ps", name="pt")
                    po = pt[:, :W2[c]]
                    for hk in range(HK):
                        nc.tensor.matmul(po, a2t[:, c, hk, ts(dm, 128)],
                                         h2[:, hk, o2[c]:o2[c] + W2[c]],
                                         start=(hk == 0), stop=(hk == HK - 1))
                    _copy_out(ci * KD + dm,
                              outc[:, dm, o2[c]:o2[c] + W2[c]], po)
            nc.sync.dma_start(yc[:], outc[:])

        # PE warmup: the HAM clock gate keeps the PE at 1.2 GHz until it has
        # been busy ~3.4us. Small N=128 matmuls on zeros keep the PE busy
        # until the first real data lands, so the real stream runs at 2.4 GHz.
        warm = wp.tile([128, TN], bf16)
        nc.gpsimd.memset(warm[:], 0.0)
        wps = pspool.tile([128, TN], f32, tag="ps", name="warmps")
        for _ in range(WARMUP_N):
            nc.tensor.matmul(wps[:, :128], warm[:, :128], warm[:, :128],
                             start=True, stop=True)

        # patch tiles 0..2, then cls (its epilogue hides under tile 3)
        for t in range(NT - 1):
            if t == 0:
                xt = xt0
            elif t == 1:
                xt = xt1  # load deferred to a t0 gelu slot
            else:
                xt = xpool.tile([128, KD, TN], bf16, tag="xt", name="xt")
                nc.sync.dma_start(xt[:], xp[:, :, ts(t, TN)])
            patch_tile(t, xt)
        cls_phase()
        xt = xpool.tile([128, KD, TN], bf16, tag="xt", name="xt")
        nc.sync.dma_start(xt[:], xp[:, :, ts(NT - 1, TN)])
        patch_tile(NT - 1, xt)

    nc.compile()
    _NC_CACHE[sig] = nc
    return nc


# ---------------------------------------------------------------- host glue
def _bf(a):
    return np.ascontiguousarray(np.asarray(a), dtype=ml_dtypes.bfloat16)


def _f32(a):
    return np.ascontiguousarray(np.asarray(a), dtype=np.float32)


def _gates(x, G_W):
    """Mirror the reference's softmax/top-1/renorm gating in fp32 on host."""
    cls_tokens = np.asarray(x[:, :NCLS], dtype=np.float32)
    logits = np.einsum("bid,ide->bie", cls_tokens, np.asarray(G_W, np.float32))
    m = logits.max(-1, keepdims=True)
    e = np.exp(logits - m)
    gate = e / e.sum(-1, keepdims=True)
    thr = np.sort(gate, axis=-1)[..., -2]
    mask = (gate > thr[..., None]).astype(np.float32)
    g = gate * mask
    g = g / np.clip(g.sum(-1, keepdims=True), 1e-6, None)
    return g  # [B, NCLS, 2], entries exactly 0.0 or 1.0 (or 0/0 on exact ties)


def _route(g):
    """Top-1 routing: bucket (batch, class) tokens by selected (atom1, atom2)."""
    buckets = {p: [] for p in ORD_PAIRS}
    for i in range(NCLS):
        for e in range(2):
            a, c = PAIRS[i][e]
            for b in np.nonzero(g[:, i, e] == 1.0)[0]:
                buckets[(a, c)].append((int(b), i))
    sig = tuple(((len(buckets[p]) + 1) // 2) * 2 for p in ORD_PAIRS)
    if sum(sig) == 0:  # all-ties degenerate case: keep tensor shapes nonzero
        sig = (2,) + (0,) * (len(ORD_PAIRS) - 1)
    return buckets, sig


def _shard_inputs(x, moe0_W1, moe0_b1, moe0_W2, moe0_b2, A1_W, A1_b, A2_W,
                  A2_b, buckets, sig):
    x = np.asarray(x, np.float32)
    Np, W1, W2, o1, o2, l1off, l2off, NCp = _cls_layout(sig)

    # shared (replicated) tensors
    # [d, h] -> [p, hm, k, c] with d = k*128+p, h = hm*128+c
    w1v = _bf(np.asarray(moe0_W1, np.float32)).reshape(KD, 128, MH, 128)
    w1v = np.ascontiguousarray(w1v.transpose(1, 2, 0, 3))
    w2v = _bf(np.asarray(moe0_W2, np.float32)).reshape(MH, 128, D).transpose(1, 0, 2)
    w2v = np.ascontiguousarray(w2v)
    b1v = np.ascontiguousarray(_f32(moe0_b1).reshape(MH, 128).T)
    b2v = np.ascontiguousarray(_f32(moe0_b2).reshape(KD, 128).T)

    # routed cls token columns, atom1-major order -> [128, KD, NCp] bf16
    xc_cols = np.zeros((max(NCp, 1), D), np.float32)
    col = 0
    for (a, c) in ORD_PAIRS:
        for (b, i) in buckets[(a, c)]:
            xc_cols[col] = x[b, i]
            col += 1
        col += Np[(a, c)] - len(buckets[(a, c)])
    xcv = _bf(xc_cols[:NCp].T.reshape(KD, 128, NCp).transpose(1, 0, 2))

    A1_W = np.asarray(A1_W, np.float32)
    A2_W = np.asarray(A2_W, np.float32)
    A1_b = np.asarray(A1_b, np.float32)

    in_maps = []
    for core in range(NCORES):
        hs = slice(core * HS, (core + 1) * HS)
        # per-core patch tokens, transposed: [128, KD, TPC]
        xpc = x[core * BPC:(core + 1) * BPC, NCLS:, :].reshape(TPC, D)
        xpv = _bf(xpc.T.reshape(KD, 128, TPC).transpose(1, 0, 2))
        # atom slices
        a1v = _bf(A1_W[:, :, hs].reshape(5, KD, 128, HS).transpose(0, 2, 1, 3))
        a2v = _bf(A2_W[:, hs, :].reshape(5, HK, 128, D).transpose(0, 2, 1, 3))
        a1bv = np.ascontiguousarray(
            A1_b[:, hs].reshape(5, HK, 128).transpose(2, 0, 1))
        in_maps.append({
            "xp": xpv, "w1": w1v, "w2": w2v, "b1": b1v, "b2": b2v,
            "xc": xcv, "a1": a1v, "a1b": a1bv, "a2": a2v,
        })
    return in_maps


def _combine_outputs(results, g, A2_b, buckets, sig):
    A2_b = np.asarray(A2_b, np.float32)
    Np, W1, W2, o1, o2, l1off, l2off, NCp = _cls_layout(sig)
    out = np.empty((B, NCLS + PT, D), np.float32)
    for core in range(NCORES):
        ypv = results[core]["yp"]  # [128, KD, TPC]
        out[core * BPC:(core + 1) * BPC, NCLS:, :] = (
            ypv.transpose(2, 1, 0).reshape(BPC, PT, D))

    cls_out = np.zeros((B, NCLS, D), np.float64)
    if NCp:
        ycs = np.zeros((128, KD, NCp), np.float64)
        for core in range(NCORES):
            ycs += results[core]["yc"]
        # yc columns are atom2-major; scatter back to (b, i)
        for c in range(5):
            for a in P2[c]:
                base = o2[c] + l2off[(a, c)]
                for idx, (b, i) in enumerate(buckets[(a, c)]):
                    cls_out[b, i] = ycs[:, :, base + idx].T.reshape(D)
    cls_out = cls_out.astype(np.float32)

    # gated atom2 bias term (biases are added pre-gating in the reference)
    a2b_sel = np.stack([[A2_b[p[1]] for p in PAIRS[i]] for i in range(NCLS)])
    cls_out += np.einsum("bie,ied->bid", g, a2b_sel).astype(np.float32)
    out[:, :NCLS, :] = cls_out
    return out


def _run(inputs, trace=False, trace_kwargs=None):
    from concourse.bass_utils import run_bass_kernel_spmd

    g = _gates(inputs["x"], inputs["G_W"])
    buckets, sig = _route(g)
    nc = _build_nc(sig)
    in_maps = _shard_inputs(
        inputs["x"], inputs["moe0_W1"], inputs["moe0_b1"], inputs["moe0_W2"],
        inputs["moe0_b2"], inputs["A1_W"], inputs["A1_b"], inputs["A2_W"],
        inputs["A2_b"], buckets, sig)
    res = run_bass_kernel_spmd(nc, in_maps, core_ids=list(range(NCORES)),
                               trace=trace, **(trace_kwargs or {}))
    out = _combine_outputs(res.results, g, inputs["A2_b"], buckets, sig)
    return out, res


def kernel(**inputs) -> np.ndarray:
    out, _ = _run(inputs, trace=False)
    return out
